# revision 12
# baseline (speedup 1.0000x reference)
"""AttnBlock (GroupNorm -> qkv 1x1 conv -> 8-head attention over 32x32
spatial -> proj 1x1 conv -> residual) on 8 Trainium2 NeuronCores.

Sharding: fully data-parallel, no collectives. Core i handles batch
b = i//2 and query-half s = i%2 (512 of the 1024 spatial positions).
Each core redundantly computes GroupNorm stats plus the full k/v
projections for its batch, then scores/softmax/AV/proj for its query
half. Host concatenates the per-core [512, 512] outputs.

v3 orchestration:
  - Input DMA on three rings: sync HWDGE carries kvf (4 tiles, gates
    GN) then xs/xo; the ACT HWDGE ring carries only wk/wv/wq (host
    prepacked to [128, 2048] so each weight is one contiguous 512KB
    transfer) and is free for activations from ~10us; gpsimd SWDGE
    carries the constants and wp.
  - x is sent as bf16 (quantizing the residual costs ~3e-3 rel err,
    threshold is 2e-2) halving the x-side DMA bytes.
  - PE warm-up matmuls bridge the DMA phase so HAM stays at 8/8.
  - k/q/v projection emits for m>=1 drip into attention tile 0/1's mk
    loop; per-mk cadence is set by the scalar-engine Exp (~1.1us), the
    PE fills its slack with the interleaved projection matmuls.
  - softmax denominators are handled per tile: the ones-column in v^T
    puts Z in psum row 64; each tile's Ln/Exp/expander-matmul/multiply
    chain is scheduled two mk-steps into the NEXT tile so the scalar
    queue never head-of-line blocks on the z DMAs (the v2 mistake that
    cost a 4.6us ACT bubble plus a HAM re-throttle).
  - proj is k-major: m=0,1 accumulate in ps_mm from tile-1's epilogue
    on; m=2,3 run at the end in freed score-psum banks.

Toolchain workarounds: the Tile-tail Drain and any instruction carrying
more than one semaphore wait are rejected by this walrus build, so
excess waits are spread onto same-engine NoOps post-schedule.
"""

import os

import numpy as np

import concourse.bass as bass
import concourse.tile as tile
from concourse import mybir
from concourse.bass_utils import run_bass_kernel_spmd
from concourse.vector_clock import ScopedClock

# ---------------------------------------------------------------------------
# walrus workaround: the Tile kernel-tail Drain may carry more sem waits than
# the CTRL instruction encoding allows; spread them over sync-engine NOPs.
_MAX_WAITS_PER_INST = 1


def _patched_drain_and_barrier(self, tick_clock, wait_clock):
    nc = self.nc
    probe = nc.sync.nop(nofuse=True, hint="drain_wait_spread")
    wait_clock.add_sem_waits(probe.ins, ScopedClock({None: tick_clock.global_clock}))
    si = probe.ins.sync_info
    waits = list(si.on_wait) if si is not None else []
    if len(waits) > _MAX_WAITS_PER_INST:
        probe.ins.sync_info = mybir.SyncInfo(
            on_wait=waits[:_MAX_WAITS_PER_INST], on_update=[]
        )
        for i in range(_MAX_WAITS_PER_INST, len(waits), _MAX_WAITS_PER_INST):
            nop = nc.sync.nop(nofuse=True, hint="drain_wait_spread")
            nop.ins.sync_info = mybir.SyncInfo(
                on_wait=waits[i : i + _MAX_WAITS_PER_INST], on_update=[]
            )
    nc.sync.drain()
    nc.all_engine_barrier(sem_only=True)
    popped = nc._tile_sem_poison_stack.pop()
    assert popped is self._sem_poison
    nc.clear_and_free_semaphores(list(self.sems.allocated().values()))


tile.TileContext._drain_and_barrier = _patched_drain_and_barrier


def _split_multi_waits(nc, max_waits=1):
    """walrus rejects instructions with more than one sem wait; move the
    excess onto same-engine NoOps placed immediately before."""
    ctr = 0
    for blk in nc.m.functions[0].blocks:
        out = []
        for inst in blk.instructions:
            si = inst.sync_info
            waits = list(si.on_wait) if (si and si.on_wait) else []
            if len(waits) > max_waits:
                extra, keep = waits[:-max_waits], waits[-max_waits:]
                for j in range(0, len(extra), max_waits):
                    ctr += 1
                    nop = mybir.InstNoOp(name=f"I-wsplit-{ctr}")
                    nop.engine = inst.engine
                    nop.sync_info = mybir.SyncInfo(
                        on_wait=extra[j : j + max_waits], on_update=[])
                    out.append(nop)
                inst.sync_info = mybir.SyncInfo(
                    on_wait=keep,
                    on_update=list(si.on_update) if si.on_update else [])
            out.append(inst)
        blk.instructions = out
    return ctr
# ---------------------------------------------------------------------------

B = 4
C = 512
H = W = 32
HWF = 1024  # keys / full spatial
Q = 512  # queries per core (half of HWF)
NH = 8
CHD = 64  # channels per head
CT = 4  # 128-channel tiles of C
KT = 8  # 128-key tiles of HWF
GROUPS = 32
GPC = 16  # channels per group
EPS = 1e-6
F32 = mybir.dt.float32
BF16 = mybir.dt.bfloat16

_DT_NAME = os.environ.get("BASS_ATTN_DT", "bf16")
DT = {"f32": mybir.dt.float32, "bf16": mybir.dt.bfloat16,
      "f32r": mybir.dt.float32r}[_DT_NAME]

N_WARM = int(os.environ.get("BASS_ATTN_WARM", "24"))


def build_program():
    nc = bass.Bass("TRN2", target_bir_lowering=False, debug=False, num_devices=8)

    def din(name, shape, dt=F32):
        return nc.declare_dram_parameter(name, list(shape), dt, isOutput=False)

    xs_d = din("xs", [128, 4 * Q], BF16)     # host prepacked [p, (t q)]
    xo_d = din("xo", [128, 4 * Q], BF16)
    kvf_d = din("kvf", [C, HWF], BF16)
    wq_d = din("wqT", [128, 4 * C], DT)      # host prepacked [p, (k m)]
    wk_d = din("wkT", [128, 4 * C], DT)
    wv_d = din("wvT", [128, 4 * C], DT)
    wp_d = din("wpT", [128, 4 * C], DT)
    bv_d = din("bv", [C])
    cpack_d = din("cpack", [128, 36])
    e16_d = din("e16", [8, 128])
    eh2_d = din("eh2", [2, 128], DT)
    out_d = nc.declare_dram_parameter("out", [C, Q], F32, isOutput=True)

    from contextlib import ExitStack
    with tile.TileContext(nc) as tc, ExitStack() as ctx:
        cst = ctx.enter_context(tc.tile_pool(name="cst", bufs=1))
        big = ctx.enter_context(tc.tile_pool(name="big", bufs=1))
        wrk = ctx.enter_context(tc.tile_pool(name="wrk", bufs=3))
        epool = ctx.enter_context(tc.tile_pool(name="epool", bufs=4))
        ps_s = ctx.enter_context(tc.tile_pool(name="ps_s", bufs=2, space="PSUM"))
        ps_o = ctx.enter_context(tc.tile_pool(name="ps_o", bufs=1, space="PSUM"))
        ps_mm = ctx.enter_context(tc.tile_pool(name="ps_mm", bufs=2, space="PSUM"))

        # ---- constants + wp on the SWDGE ring ----
        cpk = cst.tile([128, 36], F32)
        nc.gpsimd.dma_start(cpk[:], cpack_d[:])
        bq_c, bk_c, bp_c = cpk[:, 0:4], cpk[:, 4:8], cpk[:, 8:12]
        gqs_c, gqb_c = cpk[:, 12:16], cpk[:, 16:20]
        gks_c, gkb_c = cpk[:, 20:24], cpk[:, 24:28]
        g16 = cpk[:, 28:36]
        e16 = cst.tile([8, 128], F32)
        nc.gpsimd.dma_start(e16[:], e16_d[:])
        bv_ap = bv_d[:]
        bvbc = cst.tile([128, C], F32)
        nc.gpsimd.dma_start(
            out=bvbc[:],
            in_=bass.AP(tensor=bv_ap.tensor, offset=bv_ap.offset,
                        ap=[[0, 128]] + list(bv_ap.ap)),
        )
        eh2 = cst.tile([2, 128], DT)
        nc.gpsimd.dma_start(eh2[:], eh2_d[:])
        w_sb = {}
        w_sb["p"] = big.tile([128, 4 * C], DT, name="w_p")
        nc.gpsimd.dma_start(w_sb["p"][:], wp_d[:])

        # ---- weights on the ACT HWDGE ring (one 512KB DMA each) ----
        for wd, key in ((wk_d, "k"), (wv_d, "v"), (wq_d, "q")):
            t_ = big.tile([128, 4 * C], DT, name=f"w_{key}")
            nc.scalar.dma_start(t_[:], wd[:])
            w_sb[key] = t_

        def wchunk(key, k):  # [128, C] chunk of channel rows 128k..128k+127
            return w_sb[key][:, k * C : (k + 1) * C]

        # ---- big inputs on the sync HWDGE ring: kvf first (gates GN) ----
        kvf = []
        for t in range(CT):
            kt_ = big.tile([128, HWF], BF16, name=f"kvf{t}")
            nc.sync.dma_start(
                kt_[:], kvf_d[:].rearrange("(m p) q -> m p q", p=128)[t])
            kvf.append(kt_)
        xsb = big.tile([128, 4 * Q], BF16, name="xsb")
        nc.sync.dma_start(xsb[:], xs_d[:])
        xob = big.tile([128, 4 * Q], BF16, name="xob")
        nc.sync.dma_start(xob[:], xo_d[:])

        def xs_c(t):
            return xsb[:, t * Q : (t + 1) * Q]

        def xo_c(t):
            return xob[:, t * Q : (t + 1) * Q]

        # ---- PE warm-up: junk matmuls during the DMA phase keep HAM at 8/8
        junk = cst.tile([128, 256], BF16)
        nc.vector.memset(junk[:], 0.125)
        for i in range(N_WARM):
            pw = ps_mm.tile([128, 512], F32, name=f"pw{i}", tag="mm")
            nc.tensor.matmul(pw[:, 0:256], lhsT=junk[:, 0:128], rhs=junk[:],
                             start=True, stop=True)

        # ---- groupnorm affine coefficients (a, b per channel) ----
        def gn_coeffs(statc, gam, bet, label):
            gps = ps_mm.tile([128, 512], F32, name=f"gps_{label}", tag="mm")
            nc.tensor.matmul(gps[0:8, 0:8], lhsT=g16, rhs=statc[:],
                             start=True, stop=True)
            gs = wrk.tile([8, 8], F32, name=f"gs_{label}", tag="gs")
            nc.vector.tensor_copy(gs[:], gps[0:8, 0:8])
            ms = wrk.tile([8, 8], F32, name=f"ms_{label}", tag="ms")
            nc.vector.tensor_scalar_mul(ms[:], gs[:], 1.0 / GPC)
            msq8 = wrk.tile([8, 4], F32, name=f"msq8_{label}", tag="msq8")
            nc.vector.tensor_mul(msq8[:], ms[:, 0:4], ms[:, 0:4])
            var8 = wrk.tile([8, 4], F32, name=f"var8_{label}", tag="var8")
            nc.vector.tensor_sub(var8[:], ms[:, 4:8], msq8[:])
            # rstd = exp(-0.5*ln(var+eps)) — keeps ACT on one table set
            lnv = wrk.tile([8, 4], F32, name=f"lnv_{label}", tag="lnv")
            eps8 = wrk.tile([8, 1], F32, name=f"eps8_{label}", tag="eps8")
            nc.vector.memset(eps8[:], EPS)
            nc.scalar.activation(lnv[:], var8[:],
                                 mybir.ActivationFunctionType.Ln, bias=eps8[:])
            rhs2 = wrk.tile([8, 8], F32, name=f"rhs2_{label}", tag="rhs2", bufs=1)
            nc.scalar.activation(rhs2[:, 0:4], lnv[:],
                                 mybir.ActivationFunctionType.Exp, scale=-0.5)
            nc.vector.tensor_copy(rhs2[:, 4:8], ms[:, 0:4])
            pcs = ps_mm.tile([128, 512], F32, name=f"pcs_{label}", tag="mm")
            nc.tensor.matmul(pcs[:, 0:8], lhsT=e16[:], rhs=rhs2[:],
                             start=True, stop=True)
            pc = wrk.tile([128, 8], F32, name=f"pc_{label}", tag="pc")
            nc.vector.tensor_copy(pc[:], pcs[:, 0:8])
            a = wrk.tile([128, 4], F32, name=f"a_{label}", bufs=1)
            nc.vector.tensor_mul(a[:], pc[:, 0:4], gam)
            tmpb = wrk.tile([128, 4], F32, name=f"tmpb_{label}", tag="tmpb")
            nc.vector.tensor_mul(tmpb[:], pc[:, 4:8], a[:])
            b = wrk.tile([128, 4], F32, name=f"b_{label}", bufs=1)
            nc.vector.tensor_sub(b[:], bet, tmpb[:])
            return a, b

        # kv groupnorm
        statk = wrk.tile([128, 8], F32, name="statk", bufs=1)
        for t in range(CT):
            bnst = wrk.tile([128, 2, 6], F32, name="bnst_kv", tag="bnst")
            nc.vector.bn_stats(out=bnst[:, 0, :], in_=kvf[t][:, 0:512])
            nc.vector.bn_stats(out=bnst[:, 1, :], in_=kvf[t][:, 512:1024])
            mv = wrk.tile([128, 2], F32, name="mv_kv", tag="mv")
            nc.vector.bn_aggr(out=mv[:], in_=bnst[:])
            nc.vector.tensor_copy(statk[:, t : t + 1], mv[:, 0:1])
            msq = wrk.tile([128, 1], F32, name="msq_kv", tag="msq")
            nc.vector.tensor_mul(msq[:], mv[:, 0:1], mv[:, 0:1])
            nc.vector.tensor_add(statk[:, 4 + t : 5 + t], msq[:], mv[:, 1:2])
        akv, bkv = gn_coeffs(statk, gks_c, gkb_c, "kv")

        kvn = []
        for t in range(CT):
            kn = big.tile([128, HWF], DT, name=f"kvn{t}")
            nc.vector.tensor_scalar(
                out=kn[:], in0=kvf[t][:],
                scalar1=akv[:, t : t + 1], scalar2=bkv[:, t : t + 1],
                op0=mybir.AluOpType.mult, op1=mybir.AluOpType.add)
            kvn.append(kn)

        # x groupnorm (stats over both halves)
        statx = wrk.tile([128, 8], F32, name="statx", bufs=1)
        for t in range(CT):
            bnst = wrk.tile([128, 2, 6], F32, name="bnst_x", tag="bnst")
            nc.vector.bn_stats(out=bnst[:, 0, :], in_=xs_c(t))
            nc.vector.bn_stats(out=bnst[:, 1, :], in_=xo_c(t))
            mv = wrk.tile([128, 2], F32, name="mv_x", tag="mv")
            nc.vector.bn_aggr(out=mv[:], in_=bnst[:])
            nc.vector.tensor_copy(statx[:, t : t + 1], mv[:, 0:1])
            msq = wrk.tile([128, 1], F32, name="msq_x", tag="msq")
            nc.vector.tensor_mul(msq[:], mv[:, 0:1], mv[:, 0:1])
            nc.vector.tensor_add(statx[:, 4 + t : 5 + t], msq[:], mv[:, 1:2])
        ax, bx = gn_coeffs(statx, gqs_c, gqb_c, "x")

        qin = []
        for t in range(CT):
            qt = big.tile([128, Q], DT, name=f"qin{t}")
            nc.vector.tensor_scalar(
                out=qt[:], in0=xs_c(t),
                scalar1=ax[:, t : t + 1], scalar2=bx[:, t : t + 1],
                op0=mybir.AluOpType.mult, op1=mybir.AluOpType.add)
            qin.append(qt)

        k_sb = [None] * CT
        q_sb = [None] * CT
        vT_sb = [None] * KT

        _pre_ps = [("oA", ps_o), ("oB", ps_o), ("mm", ps_mm), ("mm", ps_mm)]
        _pre_i = [0]

        def qkv_ps(name, during):
            if during:
                return ps_mm.tile([128, 512], F32, name=name, tag="mm")
            tag, pool = _pre_ps[_pre_i[0] % 4]
            _pre_i[0] += 1
            return pool.tile([128, 512], F32, name=name, tag=tag)

        def emit_k(m, during):
            kt_ = big.tile([128, HWF], DT, name=f"k{m}")
            for nh in range(2):
                ps = qkv_ps(f"psk{m}{nh}", during)
                for k in range(CT):
                    nc.tensor.matmul(
                        ps[:], lhsT=wchunk("k", k)[:, bass.ts(m, 128)],
                        rhs=kvn[k][:, bass.ts(nh, 512)],
                        start=(k == 0), stop=(k == CT - 1))
                if during:
                    nc.vector.tensor_scalar_add(
                        kt_[:, bass.ts(nh, 512)], ps[:], bk_c[:, m : m + 1])
                else:
                    nc.scalar.activation(kt_[:, bass.ts(nh, 512)], ps[:],
                                         mybir.ActivationFunctionType.Identity,
                                         bias=bk_c[:, m : m + 1])
            k_sb[m] = kt_

        def emit_q(m, during):
            ps = qkv_ps(f"psq{m}", during)
            for k in range(CT):
                nc.tensor.matmul(ps[:], lhsT=wchunk("q", k)[:, bass.ts(m, 128)],
                                 rhs=qin[k][:], start=(k == 0),
                                 stop=(k == CT - 1))
            qt = big.tile([128, Q], DT, name=f"q{m}")
            if during:
                nc.vector.tensor_scalar_add(qt[:], ps[:], bq_c[:, m : m + 1])
            else:
                nc.scalar.activation(qt[:], ps[:],
                                     mybir.ActivationFunctionType.Identity,
                                     bias=bq_c[:, m : m + 1])
            q_sb[m] = qt

        def emit_v(mt, during):
            vt = big.tile([128, NH * (CHD + 1)], DT, name=f"vT{mt}")
            ones_col = vt[:].rearrange("p (h c) -> p h c", c=CHD + 1)[
                :, :, CHD : CHD + 1]
            if DT == mybir.dt.float32r:
                ones_col = ones_col.bitcast(F32)
            nc.vector.memset(ones_col, 1.0)
            ps = qkv_ps(f"psv{mt}", during)
            for k in range(CT):
                nc.tensor.matmul(
                    ps[:], lhsT=kvn[k][:, bass.ts(mt, 128)],
                    rhs=wchunk("v", k), start=(k == 0), stop=(k == CT - 1))
            nc.vector.tensor_tensor(
                out=vt[:].rearrange("p (h c) -> p h c", c=CHD + 1)[:, :, 0:CHD],
                in0=ps[:].rearrange("p (h c) -> p h c", c=CHD),
                in1=bvbc[:].rearrange("p (h c) -> p h c", c=CHD),
                op=mybir.AluOpType.add)
            vT_sb[mt] = vt

        emit_k(0, False)
        emit_v(0, False)
        emit_v(1, False)
        emit_q(0, False)

        # work units drip-fed into the attention mk loop, one per slot
        pending = (
            [("v", 2), ("v", 3), ("v", 4), ("v", 5), ("v", 6), ("v", 7),
             ("k", 1), ("q", 1), ("k", 2), ("q", 2), ("k", 3), ("q", 3)])

        def pop_emit():
            if pending:
                item = pending.pop(0)
                if callable(item):
                    item()
                    return
                kind, i = item
                if kind == "v":
                    emit_v(i, True)
                elif kind == "k":
                    emit_k(i, True)
                else:
                    emit_q(i, True)

        # ---- attention (head pairs t: heads 2t rows 0:64, 2t+1 rows 64:128)
        osts = [None] * CT
        on_sb = [None] * CT
        proj_ps = [None] * CT

        def z_chain(t):
            # Ln/Exp on this tile's two Z rows, expand to 128 partitions,
            # normalize, then its proj contribution (m=0,1).
            rzE = wrk.tile([2, 512], F32, name=f"rzE{t}", tag="rzE", bufs=2)
            nc.sync.dma_start(rzE[0:1, :], zstq[t][64:65, :])
            nc.sync.dma_start(rzE[1:2, :], stBq[t][64:65, :])

            def _act():
                lnz = wrk.tile([2, 512], F32, name=f"lnz{t}", tag="lnz", bufs=2)
                nc.scalar.activation(lnz[:], rzE[:],
                                     mybir.ActivationFunctionType.Ln)
                rzb = wrk.tile([2, 512], DT, name=f"rzb{t}", tag="rzb", bufs=2)
                nc.scalar.activation(rzb[:], lnz[:],
                                     mybir.ActivationFunctionType.Exp,
                                     scale=-1.0)
                zps = ps_s.tile([128, 1024], F32, name=f"zps{t}", tag="s")
                nc.tensor.matmul(zps[:, 0:512], lhsT=eh2[:], rhs=rzb[:],
                                 start=True, stop=True)
                ont = big.tile([128, Q], DT, name=f"on{t}")
                nc.vector.tensor_mul(ont[:], osts[t][:], zps[:, 0:512])
                on_sb[t] = ont

            def _proj():
                if proj_ps[0] is None:
                    for m in range(2):
                        proj_ps[m] = ps_mm.tile(
                            [128, 512], F32, name=f"psp{m}", tag="mm")
                for m in range(2):
                    nc.tensor.matmul(
                        proj_ps[m][:],
                        lhsT=wchunk("p", t)[:, bass.ts(m, 128)],
                        rhs=on_sb[t][:], start=(t == 0), stop=(t == CT - 1))
            return [_act, _proj]

        zstq = [None] * CT
        stBq = [None] * CT
        for t in range(CT):
            poA = ps_o.tile([128, 512], F32, name=f"poA{t}", tag="oA")
            poB = ps_o.tile([128, 512], F32, name=f"poB{t}", tag="oB")
            for mk in range(KT):
                pss = ps_s.tile([128, 1024], F32, name=f"pss{t}{mk}", tag="s")
                nc.tensor.matmul(pss[:, 0:512],
                                 lhsT=k_sb[t][0:64, bass.ts(mk, 128)],
                                 rhs=q_sb[t][0:64, :],
                                 start=True, stop=True, tile_position=(0, 0))
                nc.tensor.matmul(pss[:, 512:1024],
                                 lhsT=k_sb[t][64:128, bass.ts(mk, 128)],
                                 rhs=q_sb[t][64:128, :],
                                 start=True, stop=True, tile_position=(64, 0))
                et = epool.tile([128, 1024], DT, name=f"e{t}{mk}", tag="e")
                nc.scalar.activation(et[:], pss[:],
                                     mybir.ActivationFunctionType.Exp,
                                     scale=float(CHD) ** -0.5)
                nc.tensor.matmul(poA[0:65, :],
                                 lhsT=vT_sb[mk][:, bass.ds(130 * t, 65)],
                                 rhs=et[:, 0:512],
                                 start=(mk == 0), stop=(mk == KT - 1))
                nc.tensor.matmul(poB[0:65, :],
                                 lhsT=vT_sb[mk][:, bass.ds(130 * t + 65, 65)],
                                 rhs=et[:, 512:1024],
                                 start=(mk == 0), stop=(mk == KT - 1))
                pop_emit()
            # evacuate AV psums; head B shifts to rows 64:128 via DMA
            ost = wrk.tile([128, 512], F32, name=f"ost{t}", tag="ost", bufs=4)
            nc.vector.tensor_copy(ost[0:64, :], poA[0:64, :])
            zst = wrk.tile([65, 512], F32, name=f"zst{t}", tag="zst", bufs=4)
            nc.vector.tensor_copy(zst[64:65, :], poA[64:65, :])
            stB = wrk.tile([128, 512], F32, name=f"stB{t}", tag="stB", bufs=4)
            nc.vector.tensor_copy(stB[0:65, :], poB[0:65, :])
            nc.sync.dma_start(ost[64:128, :], stB[0:64, :])
            osts[t] = ost
            zstq[t] = zst
            stBq[t] = stB
            # schedule this tile's z chain two mk-steps into the next tile
            # (gives the z DMAs slack so Ln never blocks the ACT queue head)
            zc = z_chain(t)
            if t < CT - 1:
                pending.extend(zc)
            else:
                for fn in zc:
                    fn()

        # proj m=2,3 in the freed score psum banks, then residual + store
        for m in range(2, CT):
            ps = ps_s.tile([128, 1024], F32, name=f"psp{m}", tag="s")
            proj_ps[m] = ps
            for k in range(CT):
                nc.tensor.matmul(
                    ps[:, 0:512], lhsT=wchunk("p", k)[:, bass.ts(m, 128)],
                    rhs=on_sb[k][:], start=(k == 0), stop=(k == CT - 1))

        for m in range(CT):
            src = proj_ps[m][:] if m < 2 else proj_ps[m][:, 0:512]
            r1 = wrk.tile([128, Q], F32, name=f"r1_{m}", tag="r1")
            nc.scalar.activation(r1[:], src,
                                 mybir.ActivationFunctionType.Identity,
                                 bias=bp_c[:, m : m + 1])
            r2 = wrk.tile([128, Q], F32, name=f"r2_{m}", tag="r2")
            nc.vector.tensor_add(r2[:], r1[:], xs_c(m))
            nc.sync.dma_start(
                out_d[:].rearrange("(m p) q -> m p q", p=128)[m], r2[:])

    _split_multi_waits(nc)
    return nc


_NC_CACHE = None
LAST_EXEC_NS = None


def _np_dt():
    if DT == mybir.dt.bfloat16:
        import ml_dtypes
        return ml_dtypes.bfloat16
    return np.float32


def _prepack_w(wT, wdt):
    # [C, C] (already transposed) -> [128, (k m)] so chunk k of the SBUF
    # tile is channel rows 128k..128k+127
    return np.ascontiguousarray(
        wT.reshape(CT, 128, C).transpose(1, 0, 2).reshape(128, CT * C)
    ).astype(wdt)


def kernel(**inputs):
    global _NC_CACHE, LAST_EXEC_NS
    import ml_dtypes
    x = np.asarray(inputs["x"], dtype=np.float32)
    kv = np.asarray(inputs["kv"], dtype=np.float32)
    wdt = _np_dt()
    wqT = _prepack_w(np.asarray(inputs["wq"], np.float32).T, wdt)
    wkT = _prepack_w(np.asarray(inputs["wk"], np.float32).T, wdt)
    wvT = _prepack_w(np.asarray(inputs["wv"], np.float32).T, wdt)
    wpT = _prepack_w(np.asarray(inputs["wproj"], np.float32).T, wdt)
    bq = np.asarray(inputs["bq"], np.float32)
    bk = np.asarray(inputs["bk"], np.float32)
    bv = np.asarray(inputs["bv"], np.float32)
    bp = np.asarray(inputs["bproj"], np.float32)
    gqs = np.asarray(inputs["gnq_scale"], np.float32)
    gqb = np.asarray(inputs["gnq_bias"], np.float32)
    gks = np.asarray(inputs["gnkv_scale"], np.float32)
    gkb = np.asarray(inputs["gnkv_bias"], np.float32)

    p = np.arange(128)
    g16 = (p[:, None] // GPC == np.arange(8)[None, :]).astype(np.float32)
    e16 = np.ascontiguousarray(g16.T)
    eh2 = (np.arange(128)[None, :] // CHD == np.arange(2)[:, None]).astype(
        _np_dt())
    cpack = np.concatenate(
        [v.reshape(4, 128).T for v in (bq, bk, bp, gqs, gqb, gks, gkb)]
        + [g16], axis=1).astype(np.float32)
    cpack = np.ascontiguousarray(cpack)

    xr = x.reshape(B, C, HWF)
    kvr = kv.reshape(B, C, HWF)

    def _prepack_x(xh):  # [C, Q] -> [128, (t q)] bf16
        return np.ascontiguousarray(
            xh.reshape(CT, 128, Q).transpose(1, 0, 2).reshape(128, CT * Q)
        ).astype(ml_dtypes.bfloat16)

    in_maps = []
    for core in range(8):
        b, s = core // 2, core % 2
        in_maps.append({
            "xs": _prepack_x(xr[b][:, s * Q : (s + 1) * Q]),
            "xo": _prepack_x(xr[b][:, (1 - s) * Q : (2 - s) * Q]),
            "kvf": np.ascontiguousarray(kvr[b]).astype(ml_dtypes.bfloat16),
            "wqT": wqT, "wkT": wkT, "wvT": wvT, "wpT": wpT,
            "bv": bv, "cpack": cpack, "e16": e16, "eh2": eh2,
        })

    if _NC_CACHE is None:
        _NC_CACHE = build_program()

    trace = os.environ.get("BASS_ATTN_TRACE", "0") == "1"
    res = run_bass_kernel_spmd(_NC_CACHE, in_maps, core_ids=list(range(8)),
                               trace=trace)
    LAST_EXEC_NS = res.exec_time_ns
    globals()["LAST_RES"] = res

    out = np.empty((B, C, HWF), np.float32)
    for core in range(8):
        b, s = core // 2, core % 2
        out[b][:, s * Q : (s + 1) * Q] = res.results[core]["out"]
    return out.reshape(B, C, H, W)


# revision 17
# speedup vs baseline: 1.1330x; 1.1330x over previous
"""AttnBlock (GroupNorm -> qkv 1x1 conv -> 8-head attention over 32x32
spatial -> proj 1x1 conv -> residual) on 8 Trainium2 NeuronCores.

Sharding: fully data-parallel, no collectives. Core i handles batch
b = i//2 and query-half s = i%2 (512 of the 1024 spatial positions).
Each core redundantly computes GroupNorm stats plus the full k/v
projections for its batch, then scores/softmax/AV/proj for its query
half. Host concatenates the per-core [512, 512] outputs.

v3 orchestration:
  - Input DMA on three rings: sync HWDGE carries kvf (4 tiles, gates
    GN) then xs/xo; the ACT HWDGE ring carries only wk/wv/wq (host
    prepacked to [128, 2048] so each weight is one contiguous 512KB
    transfer) and is free for activations from ~10us; gpsimd SWDGE
    carries the constants and wp.
  - x is sent as bf16 (quantizing the residual costs ~3e-3 rel err,
    threshold is 2e-2) halving the x-side DMA bytes.
  - PE warm-up matmuls bridge the DMA phase so HAM stays at 8/8.
  - k/q/v projection emits for m>=1 drip into attention tile 0/1's mk
    loop; per-mk cadence is set by the scalar-engine Exp (~1.1us), the
    PE fills its slack with the interleaved projection matmuls.
  - softmax denominators are handled per tile: the ones-column in v^T
    puts Z in psum row 64; each tile's Ln/Exp/expander-matmul/multiply
    chain is scheduled two mk-steps into the NEXT tile so the scalar
    queue never head-of-line blocks on the z DMAs (the v2 mistake that
    cost a 4.6us ACT bubble plus a HAM re-throttle).
  - proj is k-major: m=0,1 accumulate in ps_mm from tile-1's epilogue
    on; m=2,3 run at the end in freed score-psum banks.

Toolchain workarounds: the Tile-tail Drain and any instruction carrying
more than one semaphore wait are rejected by this walrus build, so
excess waits are spread onto same-engine NoOps post-schedule.
"""

import os

import numpy as np

import concourse.bass as bass
import concourse.tile as tile
from concourse import mybir
from concourse.bass_utils import run_bass_kernel_spmd
from concourse.vector_clock import ScopedClock

# ---------------------------------------------------------------------------
# walrus workaround: the Tile kernel-tail Drain may carry more sem waits than
# the CTRL instruction encoding allows; spread them over sync-engine NOPs.
_MAX_WAITS_PER_INST = 1


def _patched_drain_and_barrier(self, tick_clock, wait_clock):
    nc = self.nc
    probe = nc.sync.nop(nofuse=True, hint="drain_wait_spread")
    wait_clock.add_sem_waits(probe.ins, ScopedClock({None: tick_clock.global_clock}))
    si = probe.ins.sync_info
    waits = list(si.on_wait) if si is not None else []
    if len(waits) > _MAX_WAITS_PER_INST:
        probe.ins.sync_info = mybir.SyncInfo(
            on_wait=waits[:_MAX_WAITS_PER_INST], on_update=[]
        )
        for i in range(_MAX_WAITS_PER_INST, len(waits), _MAX_WAITS_PER_INST):
            nop = nc.sync.nop(nofuse=True, hint="drain_wait_spread")
            nop.ins.sync_info = mybir.SyncInfo(
                on_wait=waits[i : i + _MAX_WAITS_PER_INST], on_update=[]
            )
    nc.sync.drain()
    nc.all_engine_barrier(sem_only=True)
    popped = nc._tile_sem_poison_stack.pop()
    assert popped is self._sem_poison
    nc.clear_and_free_semaphores(list(self.sems.allocated().values()))


tile.TileContext._drain_and_barrier = _patched_drain_and_barrier


def _split_multi_waits(nc, max_waits=1):
    """walrus rejects instructions with more than one sem wait; move the
    excess onto same-engine NoOps placed immediately before."""
    ctr = 0
    for blk in nc.m.functions[0].blocks:
        out = []
        for inst in blk.instructions:
            si = inst.sync_info
            waits = list(si.on_wait) if (si and si.on_wait) else []
            if len(waits) > max_waits:
                extra, keep = waits[:-max_waits], waits[-max_waits:]
                for j in range(0, len(extra), max_waits):
                    ctr += 1
                    nop = mybir.InstNoOp(name=f"I-wsplit-{ctr}")
                    nop.engine = inst.engine
                    nop.sync_info = mybir.SyncInfo(
                        on_wait=extra[j : j + max_waits], on_update=[])
                    out.append(nop)
                inst.sync_info = mybir.SyncInfo(
                    on_wait=keep,
                    on_update=list(si.on_update) if si.on_update else [])
            out.append(inst)
        blk.instructions = out
    return ctr
# ---------------------------------------------------------------------------

B = 4
C = 512
H = W = 32
HWF = 1024  # keys / full spatial
Q = 512  # queries per core (half of HWF)
NH = 8
CHD = 64  # channels per head
CT = 4  # 128-channel tiles of C
KT = 8  # 128-key tiles of HWF
GROUPS = 32
GPC = 16  # channels per group
EPS = 1e-6
F32 = mybir.dt.float32
BF16 = mybir.dt.bfloat16

_DT_NAME = os.environ.get("BASS_ATTN_DT", "bf16")
DT = {"f32": mybir.dt.float32, "bf16": mybir.dt.bfloat16,
      "f32r": mybir.dt.float32r}[_DT_NAME]

N_WARM = int(os.environ.get("BASS_ATTN_WARM", "24"))


def build_program():
    nc = bass.Bass("TRN2", target_bir_lowering=False, debug=False, num_devices=8)

    def din(name, shape, dt=F32):
        return nc.declare_dram_parameter(name, list(shape), dt, isOutput=False)

    xs_d = din("xs", [128, 4 * Q], BF16)     # host prepacked [p, (t q)]
    xo_d = din("xo", [128, 4 * Q], BF16)
    kvf_d = din("kvf", [C, HWF], BF16)
    wq_d = din("wqT", [128, 4 * C], DT)      # host prepacked [p, (k m)]
    wk_d = din("wkT", [128, 4 * C], DT)
    wv_d = din("wvT", [128, 4 * C], DT)
    wp_d = din("wpT", [128, 4 * C], DT)
    bv_d = din("bv", [C])
    cpack_d = din("cpack", [128, 36])
    e16_d = din("e16", [8, 128])
    eh2_d = din("eh2", [2, 128], DT)
    out_d = nc.declare_dram_parameter("out", [C, Q], F32, isOutput=True)

    from contextlib import ExitStack
    with tile.TileContext(nc) as tc, ExitStack() as ctx:
        cst = ctx.enter_context(tc.tile_pool(name="cst", bufs=1))
        big = ctx.enter_context(tc.tile_pool(name="big", bufs=1))
        wrk = ctx.enter_context(tc.tile_pool(name="wrk", bufs=3))
        epool = ctx.enter_context(tc.tile_pool(name="epool", bufs=4))
        ps_s = ctx.enter_context(tc.tile_pool(name="ps_s", bufs=2, space="PSUM"))
        ps_o = ctx.enter_context(tc.tile_pool(name="ps_o", bufs=1, space="PSUM"))
        ps_mm = ctx.enter_context(tc.tile_pool(name="ps_mm", bufs=2, space="PSUM"))

        # ---- constants + wp on the SWDGE ring ----
        cpk = cst.tile([128, 36], F32)
        nc.gpsimd.dma_start(cpk[:], cpack_d[:])
        bq_c, bk_c, bp_c = cpk[:, 0:4], cpk[:, 4:8], cpk[:, 8:12]
        gqs_c, gqb_c = cpk[:, 12:16], cpk[:, 16:20]
        gks_c, gkb_c = cpk[:, 20:24], cpk[:, 24:28]
        g16 = cpk[:, 28:36]
        e16 = cst.tile([8, 128], F32)
        nc.gpsimd.dma_start(e16[:], e16_d[:])
        bv_ap = bv_d[:]
        bvbc = cst.tile([128, C], F32)
        nc.gpsimd.dma_start(
            out=bvbc[:],
            in_=bass.AP(tensor=bv_ap.tensor, offset=bv_ap.offset,
                        ap=[[0, 128]] + list(bv_ap.ap)),
        )
        eh2 = cst.tile([2, 128], DT)
        nc.gpsimd.dma_start(eh2[:], eh2_d[:])
        w_sb = {}
        w_sb["p"] = big.tile([128, 4 * C], DT, name="w_p")
        nc.gpsimd.dma_start(w_sb["p"][:], wp_d[:])

        # ---- weights on the ACT HWDGE ring (one 512KB DMA each) ----
        for wd, key in ((wk_d, "k"), (wv_d, "v"), (wq_d, "q")):
            t_ = big.tile([128, 4 * C], DT, name=f"w_{key}")
            nc.scalar.dma_start(t_[:], wd[:])
            w_sb[key] = t_

        def wchunk(key, k):  # [128, C] chunk of channel rows 128k..128k+127
            return w_sb[key][:, k * C : (k + 1) * C]

        # ---- big inputs on the sync HWDGE ring: kvf first (gates GN) ----
        kvf = []
        for t in range(CT):
            kt_ = big.tile([128, HWF], BF16, name=f"kvf{t}")
            nc.sync.dma_start(
                kt_[:], kvf_d[:].rearrange("(m p) q -> m p q", p=128)[t])
            kvf.append(kt_)
        xsb = big.tile([128, 4 * Q], BF16, name="xsb")
        nc.sync.dma_start(xsb[:], xs_d[:])
        xob = big.tile([128, 4 * Q], BF16, name="xob")
        nc.sync.dma_start(xob[:], xo_d[:])

        def xs_c(t):
            return xsb[:, t * Q : (t + 1) * Q]

        def xo_c(t):
            return xob[:, t * Q : (t + 1) * Q]

        # ---- preload the Ln/Exp ACT table set during the DMA phase ----
        junkf = cst.tile([8, 2], F32)
        nc.vector.memset(junkf[:], 1.0)
        junkl = cst.tile([8, 2], F32)
        nc.scalar.activation(junkl[:, 0:1], junkf[:, 0:1],
                             mybir.ActivationFunctionType.Ln)

        # ---- PE warm-up: junk matmuls during the DMA phase keep HAM at 8/8
        junk = cst.tile([128, 256], BF16)
        nc.vector.memset(junk[:], 0.125)
        for i in range(N_WARM):
            pw = ps_mm.tile([128, 512], F32, name=f"pw{i}", tag="mm")
            nc.tensor.matmul(pw[:, 0:256], lhsT=junk[:, 0:128], rhs=junk[:],
                             start=True, stop=True)

        # ---- groupnorm affine coefficients (a, b per channel) ----
        def gn_coeffs(statc, gam, bet, label):
            gps = ps_mm.tile([128, 512], F32, name=f"gps_{label}", tag="mm")
            nc.tensor.matmul(gps[0:8, 0:8], lhsT=g16, rhs=statc[:],
                             start=True, stop=True)
            gs = wrk.tile([8, 8], F32, name=f"gs_{label}", tag="gs")
            nc.vector.tensor_copy(gs[:], gps[0:8, 0:8])
            ms = wrk.tile([8, 8], F32, name=f"ms_{label}", tag="ms")
            nc.vector.tensor_scalar_mul(ms[:], gs[:], 1.0 / GPC)
            msq8 = wrk.tile([8, 4], F32, name=f"msq8_{label}", tag="msq8")
            nc.vector.tensor_mul(msq8[:], ms[:, 0:4], ms[:, 0:4])
            var8 = wrk.tile([8, 4], F32, name=f"var8_{label}", tag="var8")
            nc.vector.tensor_sub(var8[:], ms[:, 4:8], msq8[:])
            # rstd = exp(-0.5*ln(var+eps)) — keeps ACT on one table set
            lnv = wrk.tile([8, 4], F32, name=f"lnv_{label}", tag="lnv")
            eps8 = wrk.tile([8, 1], F32, name=f"eps8_{label}", tag="eps8")
            nc.vector.memset(eps8[:], EPS)
            nc.scalar.activation(lnv[:], var8[:],
                                 mybir.ActivationFunctionType.Ln, bias=eps8[:])
            rhs2 = wrk.tile([8, 8], F32, name=f"rhs2_{label}", tag="rhs2", bufs=1)
            nc.scalar.activation(rhs2[:, 0:4], lnv[:],
                                 mybir.ActivationFunctionType.Exp, scale=-0.5)
            nc.vector.tensor_copy(rhs2[:, 4:8], ms[:, 0:4])
            pcs = ps_mm.tile([128, 512], F32, name=f"pcs_{label}", tag="mm")
            nc.tensor.matmul(pcs[:, 0:8], lhsT=e16[:], rhs=rhs2[:],
                             start=True, stop=True)
            pc = wrk.tile([128, 8], F32, name=f"pc_{label}", tag="pc")
            nc.vector.tensor_copy(pc[:], pcs[:, 0:8])
            a = wrk.tile([128, 4], F32, name=f"a_{label}", bufs=1)
            nc.vector.tensor_mul(a[:], pc[:, 0:4], gam)
            tmpb = wrk.tile([128, 4], F32, name=f"tmpb_{label}", tag="tmpb")
            nc.vector.tensor_mul(tmpb[:], pc[:, 4:8], a[:])
            b = wrk.tile([128, 4], F32, name=f"b_{label}", bufs=1)
            nc.vector.tensor_sub(b[:], bet, tmpb[:])
            return a, b

        # kv groupnorm
        statk = wrk.tile([128, 8], F32, name="statk", bufs=1)
        for t in range(CT):
            bnst = wrk.tile([128, 2, 6], F32, name="bnst_kv", tag="bnst")
            nc.vector.bn_stats(out=bnst[:, 0, :], in_=kvf[t][:, 0:512])
            nc.vector.bn_stats(out=bnst[:, 1, :], in_=kvf[t][:, 512:1024])
            mv = wrk.tile([128, 2], F32, name="mv_kv", tag="mv")
            nc.vector.bn_aggr(out=mv[:], in_=bnst[:])
            nc.vector.tensor_copy(statk[:, t : t + 1], mv[:, 0:1])
            msq = wrk.tile([128, 1], F32, name="msq_kv", tag="msq")
            nc.vector.tensor_mul(msq[:], mv[:, 0:1], mv[:, 0:1])
            nc.vector.tensor_add(statk[:, 4 + t : 5 + t], msq[:], mv[:, 1:2])
        akv, bkv = gn_coeffs(statk, gks_c, gkb_c, "kv")

        kvn = []
        for t in range(CT):
            kn = big.tile([128, HWF], DT, name=f"kvn{t}")
            nc.vector.tensor_scalar(
                out=kn[:], in0=kvf[t][:],
                scalar1=akv[:, t : t + 1], scalar2=bkv[:, t : t + 1],
                op0=mybir.AluOpType.mult, op1=mybir.AluOpType.add)
            kvn.append(kn)

        # x groupnorm (stats over both halves)
        statx = wrk.tile([128, 8], F32, name="statx", bufs=1)
        for t in range(CT):
            bnst = wrk.tile([128, 2, 6], F32, name="bnst_x", tag="bnst")
            nc.vector.bn_stats(out=bnst[:, 0, :], in_=xs_c(t))
            nc.vector.bn_stats(out=bnst[:, 1, :], in_=xo_c(t))
            mv = wrk.tile([128, 2], F32, name="mv_x", tag="mv")
            nc.vector.bn_aggr(out=mv[:], in_=bnst[:])
            nc.vector.tensor_copy(statx[:, t : t + 1], mv[:, 0:1])
            msq = wrk.tile([128, 1], F32, name="msq_x", tag="msq")
            nc.vector.tensor_mul(msq[:], mv[:, 0:1], mv[:, 0:1])
            nc.vector.tensor_add(statx[:, 4 + t : 5 + t], msq[:], mv[:, 1:2])
        ax, bx = gn_coeffs(statx, gqs_c, gqb_c, "x")

        qin = []
        for t in range(CT):
            qt = big.tile([128, Q], DT, name=f"qin{t}")
            nc.vector.tensor_scalar(
                out=qt[:], in0=xs_c(t),
                scalar1=ax[:, t : t + 1], scalar2=bx[:, t : t + 1],
                op0=mybir.AluOpType.mult, op1=mybir.AluOpType.add)
            qin.append(qt)

        k_sb = [None] * CT
        q_sb = [None] * CT
        vT_sb = [None] * KT

        _pre_ps = [("oA", ps_o), ("oB", ps_o), ("mm", ps_mm), ("mm", ps_mm)]
        _pre_i = [0]

        def qkv_ps(name, during):
            if during:
                return ps_mm.tile([128, 512], F32, name=name, tag="mm")
            tag, pool = _pre_ps[_pre_i[0] % 4]
            _pre_i[0] += 1
            return pool.tile([128, 512], F32, name=name, tag=tag)

        def emit_k(m, during):
            kt_ = big.tile([128, HWF], DT, name=f"k{m}")
            for nh in range(2):
                ps = qkv_ps(f"psk{m}{nh}", during)
                for k in range(CT):
                    nc.tensor.matmul(
                        ps[:], lhsT=wchunk("k", k)[:, bass.ts(m, 128)],
                        rhs=kvn[k][:, bass.ts(nh, 512)],
                        start=(k == 0), stop=(k == CT - 1))
                if during:
                    nc.vector.tensor_scalar_add(
                        kt_[:, bass.ts(nh, 512)], ps[:], bk_c[:, m : m + 1])
                else:
                    nc.scalar.activation(kt_[:, bass.ts(nh, 512)], ps[:],
                                         mybir.ActivationFunctionType.Identity,
                                         bias=bk_c[:, m : m + 1])
            k_sb[m] = kt_

        def emit_q(m, during):
            ps = qkv_ps(f"psq{m}", during)
            for k in range(CT):
                nc.tensor.matmul(ps[:], lhsT=wchunk("q", k)[:, bass.ts(m, 128)],
                                 rhs=qin[k][:], start=(k == 0),
                                 stop=(k == CT - 1))
            qt = big.tile([128, Q], DT, name=f"q{m}")
            if during:
                nc.vector.tensor_scalar_add(qt[:], ps[:], bq_c[:, m : m + 1])
            else:
                nc.scalar.activation(qt[:], ps[:],
                                     mybir.ActivationFunctionType.Identity,
                                     bias=bq_c[:, m : m + 1])
            q_sb[m] = qt

        def emit_v(mt, during):
            vt = big.tile([128, NH * (CHD + 1)], DT, name=f"vT{mt}")
            ones_col = vt[:].rearrange("p (h c) -> p h c", c=CHD + 1)[
                :, :, CHD : CHD + 1]
            if DT == mybir.dt.float32r:
                ones_col = ones_col.bitcast(F32)
            nc.vector.memset(ones_col, 1.0)
            ps = qkv_ps(f"psv{mt}", during)
            for k in range(CT):
                nc.tensor.matmul(
                    ps[:], lhsT=kvn[k][:, bass.ts(mt, 128)],
                    rhs=wchunk("v", k), start=(k == 0), stop=(k == CT - 1))
            nc.vector.tensor_tensor(
                out=vt[:].rearrange("p (h c) -> p h c", c=CHD + 1)[:, :, 0:CHD],
                in0=ps[:].rearrange("p (h c) -> p h c", c=CHD),
                in1=bvbc[:].rearrange("p (h c) -> p h c", c=CHD),
                op=mybir.AluOpType.add)
            vT_sb[mt] = vt

        emit_k(0, False)
        emit_v(0, False)
        emit_v(1, False)
        emit_q(0, False)

        # work units drip-fed into the attention mk loop, one per slot
        pending = (
            [("v", 2), ("v", 3), ("v", 4), ("v", 5), ("v", 6), ("v", 7),
             ("k", 1), ("q", 1), ("k", 2), ("q", 2), ("k", 3), ("q", 3)])

        def pop_emit():
            if pending:
                item = pending.pop(0)
                if callable(item):
                    item()
                    return
                kind, i = item
                if kind == "v":
                    emit_v(i, True)
                elif kind == "k":
                    emit_k(i, True)
                else:
                    emit_q(i, True)

        # ---- attention (head pairs t: heads 2t rows 0:64, 2t+1 rows 64:128)
        osts = [None] * CT
        on_sb = [None] * CT
        proj_ps = [None] * CT

        def z_chain(t, rzE):
            # Ln/Exp on this tile's two Z rows, expand to 128 partitions,
            # normalize, then its proj contribution (m=0,1).
            def _act():
                lnz = wrk.tile([2, 512], F32, name=f"lnz{t}", tag="lnz", bufs=2)
                nc.scalar.activation(lnz[:], rzE[:],
                                     mybir.ActivationFunctionType.Ln)
                rzb = wrk.tile([2, 512], DT, name=f"rzb{t}", tag="rzb", bufs=2)
                nc.scalar.activation(rzb[:], lnz[:],
                                     mybir.ActivationFunctionType.Exp,
                                     scale=-1.0)
                zps = ps_s.tile([128, 1024], F32, name=f"zps{t}", tag="s")
                nc.tensor.matmul(zps[:, 0:512], lhsT=eh2[:], rhs=rzb[:],
                                 start=True, stop=True)
                ont = big.tile([128, Q], DT, name=f"on{t}")
                nc.vector.tensor_mul(ont[:], osts[t][:], zps[:, 0:512])
                on_sb[t] = ont

            def _proj():
                if proj_ps[0] is None:
                    for m in range(2):
                        proj_ps[m] = ps_mm.tile(
                            [128, 512], F32, name=f"psp{m}", tag="mm")
                for m in range(2):
                    nc.tensor.matmul(
                        proj_ps[m][:],
                        lhsT=wchunk("p", t)[:, bass.ts(m, 128)],
                        rhs=on_sb[t][:], start=(t == 0), stop=(t == CT - 1))
            return [_act, _proj]

        stBq = [None] * CT
        for t in range(CT):
            poA = ps_o.tile([128, 512], F32, name=f"poA{t}", tag="oA")
            poB = ps_o.tile([128, 512], F32, name=f"poB{t}", tag="oB")
            for mk in range(KT):
                pss = ps_s.tile([128, 1024], F32, name=f"pss{t}{mk}", tag="s")
                nc.tensor.matmul(pss[:, 0:512],
                                 lhsT=k_sb[t][0:64, bass.ts(mk, 128)],
                                 rhs=q_sb[t][0:64, :],
                                 start=True, stop=True, tile_position=(0, 0))
                nc.tensor.matmul(pss[:, 512:1024],
                                 lhsT=k_sb[t][64:128, bass.ts(mk, 128)],
                                 rhs=q_sb[t][64:128, :],
                                 start=True, stop=True, tile_position=(64, 0))
                et = epool.tile([128, 1024], DT, name=f"e{t}{mk}", tag="e")
                nc.scalar.activation(et[:], pss[:],
                                     mybir.ActivationFunctionType.Exp,
                                     scale=float(CHD) ** -0.5)
                nc.tensor.matmul(poA[0:65, :],
                                 lhsT=vT_sb[mk][:, bass.ds(130 * t, 65)],
                                 rhs=et[:, 0:512],
                                 start=(mk == 0), stop=(mk == KT - 1))
                nc.tensor.matmul(poB[0:65, :],
                                 lhsT=vT_sb[mk][:, bass.ds(130 * t + 65, 65)],
                                 rhs=et[:, 512:1024],
                                 start=(mk == 0), stop=(mk == KT - 1))
                pop_emit()
            # evacuate AV psums; head B shifts to rows 64:128 via DMA.
            # Z_A rides in ost row 64 and is DMA'd out to rzE before the
            # shift overwrites that row (sync-queue FIFO orders the two).
            ost = wrk.tile([128, 512], F32, name=f"ost{t}", tag="ost", bufs=4)
            nc.vector.tensor_copy(ost[0:65, :], poA[0:65, :])
            stB = wrk.tile([128, 512], F32, name=f"stB{t}", tag="stB", bufs=4)
            nc.vector.tensor_copy(stB[0:65, :], poB[0:65, :])
            rzE = wrk.tile([2, 512], F32, name=f"rzE{t}", tag="rzE", bufs=2)
            nc.sync.dma_start(rzE[0:1, :], ost[64:65, :])
            nc.sync.dma_start(rzE[1:2, :], stB[64:65, :])
            nc.sync.dma_start(ost[64:128, :], stB[0:64, :])
            osts[t] = ost
            stBq[t] = stB
            # schedule this tile's z chain four mk-steps into the next tile
            # (gives the z DMAs slack so Ln never blocks the ACT queue head)
            zc = z_chain(t, rzE)
            if t < CT - 1:
                while len(pending) < 4:
                    pending.append(lambda: None)
                pending.extend(zc)
            else:
                for fn in zc:
                    fn()

        # proj m=2,3 in the freed score psum banks, then residual + store
        for m in range(2, CT):
            ps = ps_s.tile([128, 1024], F32, name=f"psp{m}", tag="s")
            proj_ps[m] = ps
            for k in range(CT):
                nc.tensor.matmul(
                    ps[:, 0:512], lhsT=wchunk("p", k)[:, bass.ts(m, 128)],
                    rhs=on_sb[k][:], start=(k == 0), stop=(k == CT - 1))

        for m in range(CT):
            src = proj_ps[m][:] if m < 2 else proj_ps[m][:, 0:512]
            r1 = wrk.tile([128, Q], F32, name=f"r1_{m}", tag="r1")
            nc.scalar.activation(r1[:], src,
                                 mybir.ActivationFunctionType.Identity,
                                 bias=bp_c[:, m : m + 1])
            r2 = wrk.tile([128, Q], F32, name=f"r2_{m}", tag="r2")
            nc.vector.tensor_add(r2[:], r1[:], xs_c(m))
            nc.sync.dma_start(
                out_d[:].rearrange("(m p) q -> m p q", p=128)[m], r2[:])

    _split_multi_waits(nc)
    return nc


_NC_CACHE = None
LAST_EXEC_NS = None


def _np_dt():
    if DT == mybir.dt.bfloat16:
        import ml_dtypes
        return ml_dtypes.bfloat16
    return np.float32


def _prepack_w(wT, wdt):
    # [C, C] (already transposed) -> [128, (k m)] so chunk k of the SBUF
    # tile is channel rows 128k..128k+127
    return np.ascontiguousarray(
        wT.reshape(CT, 128, C).transpose(1, 0, 2).reshape(128, CT * C)
    ).astype(wdt)


def kernel(**inputs):
    global _NC_CACHE, LAST_EXEC_NS
    import ml_dtypes
    x = np.asarray(inputs["x"], dtype=np.float32)
    kv = np.asarray(inputs["kv"], dtype=np.float32)
    wdt = _np_dt()
    wqT = _prepack_w(np.asarray(inputs["wq"], np.float32).T, wdt)
    wkT = _prepack_w(np.asarray(inputs["wk"], np.float32).T, wdt)
    wvT = _prepack_w(np.asarray(inputs["wv"], np.float32).T, wdt)
    wpT = _prepack_w(np.asarray(inputs["wproj"], np.float32).T, wdt)
    bq = np.asarray(inputs["bq"], np.float32)
    bk = np.asarray(inputs["bk"], np.float32)
    bv = np.asarray(inputs["bv"], np.float32)
    bp = np.asarray(inputs["bproj"], np.float32)
    gqs = np.asarray(inputs["gnq_scale"], np.float32)
    gqb = np.asarray(inputs["gnq_bias"], np.float32)
    gks = np.asarray(inputs["gnkv_scale"], np.float32)
    gkb = np.asarray(inputs["gnkv_bias"], np.float32)

    p = np.arange(128)
    g16 = (p[:, None] // GPC == np.arange(8)[None, :]).astype(np.float32)
    e16 = np.ascontiguousarray(g16.T)
    eh2 = (np.arange(128)[None, :] // CHD == np.arange(2)[:, None]).astype(
        _np_dt())
    cpack = np.concatenate(
        [v.reshape(4, 128).T for v in (bq, bk, bp, gqs, gqb, gks, gkb)]
        + [g16], axis=1).astype(np.float32)
    cpack = np.ascontiguousarray(cpack)

    xr = x.reshape(B, C, HWF)
    kvr = kv.reshape(B, C, HWF)

    def _prepack_x(xh):  # [C, Q] -> [128, (t q)] bf16
        return np.ascontiguousarray(
            xh.reshape(CT, 128, Q).transpose(1, 0, 2).reshape(128, CT * Q)
        ).astype(ml_dtypes.bfloat16)

    in_maps = []
    for core in range(8):
        b, s = core // 2, core % 2
        in_maps.append({
            "xs": _prepack_x(xr[b][:, s * Q : (s + 1) * Q]),
            "xo": _prepack_x(xr[b][:, (1 - s) * Q : (2 - s) * Q]),
            "kvf": np.ascontiguousarray(kvr[b]).astype(ml_dtypes.bfloat16),
            "wqT": wqT, "wkT": wkT, "wvT": wvT, "wpT": wpT,
            "bv": bv, "cpack": cpack, "e16": e16, "eh2": eh2,
        })

    if _NC_CACHE is None:
        _NC_CACHE = build_program()

    trace = os.environ.get("BASS_ATTN_TRACE", "0") == "1"
    res = run_bass_kernel_spmd(_NC_CACHE, in_maps, core_ids=list(range(8)),
                               trace=trace)
    LAST_EXEC_NS = res.exec_time_ns
    globals()["LAST_RES"] = res

    out = np.empty((B, C, HWF), np.float32)
    for core in range(8):
        b, s = core // 2, core % 2
        out[b][:, s * Q : (s + 1) * Q] = res.results[core]["out"]
    return out.reshape(B, C, H, W)
